# revision 14
# baseline (speedup 1.0000x reference)
"""AdderNet 2D convolution on 8 TRN2 NeuronCores.

out[n,co,h,w] = -sum_{ci,kh,kw} |xpad[n,ci,h+kh,w+kw] - w[co,ci,kh,kw]|

Sharding: data-parallel over the batch dim (16 images -> 2 per core),
weight replicated.  No collectives needed (forward pass only).

Math: |x - w| = x + w - 2*min(x, w), so

  -sum |x - w| = 2*sum min(x, w) - sum x - sum w

The heavy term is one single-op DVE tensor_scalar per (co, tap):
y = min(xpad, w[co,ci,kh,kw]) in bf16 (4x perf mode), evaluated over the
full padded plane so every instruction is contiguous/aligned.  TensorE
reduces partitions with a constant +2 block stationary into PSUM
(accumulating the 9 taps); the (kh,kw) tap shift is applied by the
matmul's strided moving-view.  "sum x" is accumulated by 252 extra
matmuls with an all-(-1) stationary; "sum w" comes in as a tiny
host-precomputed per-partition bias, applied in the epilogue.

Per-core layout:
  - 128 SBUF partitions = img*64 + ci  (2 images per core)
  - psum/output partition p = 32*(co//16) + 2*(co%16) + img
    (TensorE column-tiling: 4 strips of 32, one per co-group)
  - zero padding in xpad contributes min(0, w) terms and the matching
    zeros in sum x, exactly reproducing the reference's |0 - w| border
    terms.
"""

import numpy as np

try:
    from concourse import bacc, mybir, tile
except ImportError:  # pragma: no cover - fallback when sitecustomize absent
    import sys

    sys.path.insert(0, "/opt/trn_rl_repo")
    from concourse import bacc, mybir, tile

from concourse.bass_utils import run_bass_kernel_spmd

N, C, H, W = 16, 64, 56, 56
CO, K = 64, 3
NCORES = 8
NLOC = N // NCORES  # images per core = 2
HP = H + 2  # padded plane height
WP = W + 2
L = H * W  # 3136 output pixels
CHUNK_ROWS = 8  # output rows per psum bank chunk
NCHUNK = H // CHUNK_ROWS  # 7
CHUNK = CHUNK_ROWS * W  # 448 <= 512 fp32 / psum bank

_nc_cache = None


def build_nc():
    nc = bacc.Bacc(
        "TRN2",
        target_bir_lowering=False,
        debug=False,
        num_devices=NCORES,
    )
    f32 = mybir.dt.float32
    bf16 = mybir.dt.bfloat16

    x_d = nc.dram_tensor("x", [NLOC, C, H, W], f32, kind="ExternalInput")
    w_d = nc.dram_tensor("w", [CO, C, K, K], f32, kind="ExternalInput")
    # swn[p, 0] = -sum_{ci,kh,kw} w[co(p)] at psum partition p (host-computed)
    swn_d = nc.dram_tensor("swn", [128, 1], f32, kind="ExternalInput")
    # out rows are psum-partition-major: p = 32*(co//16) + 2*(co%16) + img;
    # the host-side gather untangles this ordering (cheap numpy transpose).
    o_d = nc.dram_tensor("out", [128, L], f32, kind="ExternalOutput")

    with tile.TileContext(nc) as tc:
        with (
            tc.tile_pool(name="const", bufs=1) as cpool,
            tc.tile_pool(name="ypool", bufs=4) as ypool,
            tc.tile_pool(name="psum", bufs=1, space="PSUM") as ppool,
        ):
            xstage = cpool.tile([128, H, W], f32)
            xpad = cpool.tile([128, HP, WP], bf16)
            wbias = cpool.tile([128, CO, K * K], f32)
            swn = cpool.tile([128, 1], f32)
            # stat2[:, c, :]: [128, 32] stationary, col 2c+i = +2 on the
            # img-i partition half, else 0  (the 2*min reduction).
            stat2 = cpool.tile([128, 16, 32], bf16)
            # statn: [128, 32] all-columns -1 on matching img half (sum-x).
            statn = cpool.tile([128, 32], bf16)
            out_sb = cpool.tile([128, L], f32)

            # ---- loads -------------------------------------------------
            nc.sync.dma_start(xstage[:], x_d.ap().rearrange("n c h w -> (n c) h w"))
            # wbias[p = img*64 + ci, co, kh*3 + kw] = w[co, ci, kh, kw]
            wv = w_d.ap().rearrange("co ci kh kw -> ci co (kh kw)")
            nc.sync.dma_start(wbias[0:64], wv)
            nc.sync.dma_start(wbias[64:128], wv)
            nc.sync.dma_start(swn[:], swn_d.ap())

            # ---- constants --------------------------------------------
            nc.vector.memset(stat2[:], 0.0)
            for c in range(16):
                nc.vector.memset(stat2[0:64, c, 2 * c : 2 * c + 1], 2.0)
                nc.vector.memset(stat2[64:128, c, 2 * c + 1 : 2 * c + 2], 2.0)
            nc.vector.memset(statn[:], 0.0)
            nc.vector.memset(statn[0:64, 0:32:2], -1.0)
            nc.vector.memset(statn[64:128, 1:32:2], -1.0)

            # ---- pad + cast to bf16 -----------------------------------
            nc.vector.memset(xpad[:], 0.0)
            nc.vector.tensor_copy(xpad[:, 1 : H + 1, 1 : W + 1], xstage[:])

            psums = [
                ppool.tile([128, CHUNK], f32, name=f"ps{f}", tag=f"ps{f}")
                for f in range(NCHUNK)
            ]

            taps = [(kh, kw) for kh in range(K) for kw in range(K)]

            # ---- sum-x accumulation (also the PE warm-up burst) --------
            for f in range(NCHUNK):
                r0 = f * CHUNK_ROWS
                for t, (kh, kw) in enumerate(taps):
                    rhs = xpad[:, r0 + kh : r0 + kh + CHUNK_ROWS, kw : kw + W]
                    for g in range(4):
                        nc.tensor.matmul(
                            psums[f][32 * g : 32 * g + 32, :],
                            statn[:],
                            rhs,
                            start=(t == 0),
                            stop=False,
                            tile_position=(0, 32 * g),
                        )

            # ---- main loop: 2*sum min(x, w) ----------------------------
            for cc in range(16):
                for t, (kh, kw) in enumerate(taps):
                    for g in range(4):
                        co = 16 * g + cc
                        y = ypool.tile([128, HP, WP], bf16, tag="y")
                        nc.vector.tensor_scalar(
                            y[:],
                            xpad[:],
                            wbias[:, co, t : t + 1],
                            None,
                            op0=mybir.AluOpType.min,
                        )
                        for f in range(NCHUNK):
                            r0 = f * CHUNK_ROWS
                            rhs = y[:, r0 + kh : r0 + kh + CHUNK_ROWS, kw : kw + W]
                            nc.tensor.matmul(
                                psums[f][32 * g : 32 * g + 32, :],
                                stat2[:, cc, :],
                                rhs,
                                start=False,
                                stop=(cc == 15 and t == len(taps) - 1),
                                tile_position=(0, 32 * g),
                            )

            # ---- epilogue: out = psum + (-sum w), psum -> sbuf -> dram --
            for f in range(NCHUNK):
                nc.scalar.activation(
                    out_sb[:, f * CHUNK : (f + 1) * CHUNK],
                    psums[f][:],
                    mybir.ActivationFunctionType.Identity,
                    bias=swn[:],
                )
            nc.sync.dma_start(o_d.ap(), out_sb[:])

    nc.compile()
    return nc


def get_nc():
    global _nc_cache
    if _nc_cache is None:
        _nc_cache = build_nc()
    return _nc_cache


def make_in_maps(x, w):
    x = np.ascontiguousarray(x, dtype=np.float32)
    w = np.ascontiguousarray(w, dtype=np.float32)
    # -sum w[co] scattered to psum partitions p = 32*(co//16)+2*(co%16)+img
    swc = -w.reshape(CO, -1).sum(axis=1)
    swn = np.empty((128, 1), dtype=np.float32)
    for co in range(CO):
        p = 32 * (co // 16) + 2 * (co % 16)
        swn[p, 0] = swc[co]
        swn[p + 1, 0] = swc[co]
    return [
        {"x": x[i * NLOC : (i + 1) * NLOC], "w": w, "swn": swn}
        for i in range(NCORES)
    ]


def unscramble(core_out):
    """[128, L] with row p = 32*(co//16) + 2*(co%16) + img -> [2, 64, 56, 56]."""
    return (
        core_out.reshape(4, 16, NLOC, H, W)
        .transpose(2, 0, 1, 3, 4)
        .reshape(NLOC, CO, H, W)
    )


def kernel(x, w):
    nc = get_nc()
    res = run_bass_kernel_spmd(nc, make_in_maps(x, w), core_ids=list(range(NCORES)))
    out = np.concatenate([unscramble(r["out"]) for r in res.results], axis=0)
    return np.ascontiguousarray(out, dtype=np.float32)


if __name__ == "__main__":
    x = np.random.randn(N, C, H, W).astype(np.float32)
    w = np.random.randn(CO, C, K, K).astype(np.float32)
    o = kernel(x, w)
    print("out", o.shape, o.dtype, float(o.mean()))
